# revision 15
# baseline (speedup 1.0000x reference)
"""MoE feed-forward (top-2 of 8 experts) on 8 TRN2 NeuronCores.

Strategy (expert-parallel, per the sharding hint):
  - Host: gate in fp64 (logits -> softmax -> top-2), pack each expert's
    routed tokens into a fixed-capacity buffer, one expert per core.
  - Core e (fused, no DRAM spill):
      Phase A: hT = gelu(W1[e]^T x^T + b1[e]) kept entirely in SBUF,
               produced per (f-tile, token-group) from bf16 GEMMs.
      Phase B: yT = (W2[e]^T hT) * wt, accumulated over all 44 f-tiles
               in PSUM per (d-tile, token-group); tokens ride the moving
               dim in BOTH GEMMs, so cost scales with the exact token
               capacity (no 128-token padding).
  - fp8 fast lane: per expert, the KF8 rank-2 tokens with the smallest
    softmax weight run GEMM2 in fp8-e4m3 DoubleRow (2 MACs/cell/cycle):
    h stored as e4m3, W2 pre-scaled x256 into e4m3, the 1/256 folded
    into the combine weight. Their error contribution is bounded by the
    small combine weights (verified ~1.5e-2 total vs the 2e-2 gate).
  - Host: scatter-add the 8 per-expert yT outputs into the dense result.

All other matmuls are bf16 (fp32 runs at 1/4 rate); accumulation is fp32
in PSUM. All DRAM tensors are host-packed into the exact tiled layouts
the kernel consumes, so every DMA line is large and contiguous.
"""

import os

if os.environ.get("JAX_PLATFORMS") == "cpu":
    # The bass kernel executes through the axon PJRT platform; a cpu-only
    # pin would leave no NeuronCores visible.
    os.environ["JAX_PLATFORMS"] = ""

import numpy as np
import ml_dtypes

P = 128
D = 2048
F = 5632
E = 8
TOP_K = 2
N_CORES = 8
KD = D // P  # 16 k-tiles over D (GEMM1 contraction)
FT = F // P  # 44 f-tiles (GEMM2 contraction)
FTH = FT // 2  # 22 fp8 DoubleRow pair-tiles
DT = D // P  # 16 d-tiles (GEMM2 output rows)
FBW = 256  # W1 f-columns streamed per block
NFB = F // FBW  # 22 blocks
KF8 = 416  # per-expert count of smallest-weight rank-2 tokens on fp8 lane
F8_SCALE = 256.0  # W2 pre-scale for e4m3 (power of two, folded into wt)


def _gate_host(flat, gate_w, gate_b):
    """fp64 gating: returns per-token top-k expert ids and softmax scores."""
    logits = flat.astype(np.float64) @ gate_w.astype(np.float64) + gate_b.astype(
        np.float64
    )
    m = logits.max(axis=-1, keepdims=True)
    e = np.exp(logits - m)
    s = e / e.sum(axis=-1, keepdims=True)
    # stable argsort of -s == lax.top_k tie-breaking (lowest index first)
    order = np.argsort(-s, axis=-1, kind="stable")
    top_i = order[:, :TOP_K]
    return top_i, s


def _token_groups(cap):
    """Split cap tokens into ceil(cap/512) near-equal groups (each a
    multiple of 4, <=512). Keeping every group >=256 cols hides the
    LDWEIGHTS behind the matmul stream."""
    k = -(-cap // 512)
    q = -(-cap // (4 * k)) * 4
    out = []
    t0 = 0
    while t0 < cap:
        t1 = min(t0 + q, cap)
        out.append((t0, t1))
        t0 = t1
    return out


def _build_program(capB, capF):
    import concourse.bass as bass
    import concourse.mybir as mybir
    import concourse.tile as tile

    f32 = mybir.dt.float32
    bf16 = mybir.dt.bfloat16
    f8e4 = mybir.dt.float8e4
    DR = mybir.MatmulPerfMode.DoubleRow
    groups = _token_groups(capB)
    cap = capB + capF

    nc = bass.Bass()
    xgd = [
        nc.dram_tensor(f"xg{gi}", [P, KD, n1 - n0], bf16, kind="ExternalInput")
        for gi, (n0, n1) in enumerate(groups)
    ]
    xfd = nc.dram_tensor("xf", [P, KD, capF], bf16, kind="ExternalInput")
    w1 = nc.dram_tensor("w1", [NFB, P, KD, FBW], bf16, kind="ExternalInput")
    w2 = nc.dram_tensor("w2", [DT, P, FT, P], bf16, kind="ExternalInput")
    w2f = nc.dram_tensor("w2f", [DT, P, FTH, 2, P], f8e4, kind="ExternalInput")
    b1 = nc.dram_tensor("b1", [P, FT], f32, kind="ExternalInput")
    wt = nc.dram_tensor("wt", [P, cap], f32, kind="ExternalInput")
    y = nc.dram_tensor("y", [D, cap], f32, kind="ExternalOutput")

    FL = FBW // P  # f-tiles per W1 block

    with tile.TileContext(nc) as tc:
        with (
            tc.tile_pool(name="const", bufs=1) as constp,
            tc.tile_pool(name="w2pool", bufs=3) as w2pool,
            tc.tile_pool(name="w2fpool", bufs=3) as w2fpool,
            tc.tile_pool(name="htpool", bufs=1) as htpool,
        ):
            hT = htpool.tile([P, FT, capB], bf16)
            hf = htpool.tile([P, FTH, 2, capF], f8e4)
            w2_tiles = {}
            w2f_tiles = {}

            # ---- Phase A: hT = gelu(w1.T @ x.T + b1), kept in SBUF ----
            with (
                tc.tile_pool(name="xpool", bufs=1) as xpool,
                tc.tile_pool(name="w1pool", bufs=2) as w1pool,
                tc.tile_pool(name="psA", bufs=4, space="PSUM") as psA,
            ):
                # W1 block 0 rides the ACT HWDGE queue, in parallel with
                # xg0 on the sync queue: first matmul waits only on these
                # two ~1 MB contiguous transfers.
                w1_b0 = w1pool.tile([P, KD, FBW], bf16, tag="w1sb")
                nc.scalar.dma_start(w1_b0[:], w1[0])
                # one xT tile per token group (separate dram tensors, each
                # contiguous per partition): first matmul only waits for g0
                xg = []
                for gi, (n0, n1) in enumerate(groups):
                    t = xpool.tile([P, KD, n1 - n0], bf16, name=f"xg{gi}")
                    nc.sync.dma_start(t[:], xgd[gi][:])
                    xg.append(t)
                # xf/b1/wt ride the ACT HWDGE queue, in parallel with the
                # xg group loads on sync
                xf_sb = xpool.tile([P, KD, capF], bf16, name="xf")
                nc.scalar.dma_start(xf_sb[:], xfd[:])
                b1_sb = constp.tile([P, FT], f32)
                nc.scalar.dma_start(b1_sb[:], b1[:, :])
                wt_sb = constp.tile([P, cap], f32)
                nc.scalar.dma_start(wt_sb[:], wt[:, :])

                for fb in range(NFB):
                    if fb == 0:
                        w1_sb = w1_b0
                    else:
                        w1_sb = w1pool.tile([P, KD, FBW], bf16, tag="w1sb")
                        nc.sync.dma_start(w1_sb[:], w1[fb])
                    if NFB - 6 <= fb < NFB - 3:
                        # prefetch the first 3 W2/W2f d-tiles late in
                        # phase A, on the ACT HWDGE queue so they never
                        # delay the W1/x loads on the sync queue
                        dtp = fb - (NFB - 6)
                        w2_sb = w2pool.tile([P, FT, P], bf16, tag="w2sb")
                        nc.scalar.dma_start(w2_sb[:], w2[dtp])
                        w2_tiles[dtp] = w2_sb
                        w2f_sb = w2fpool.tile([P, FTH, 2, P], f8e4, tag="w2fsb")
                        nc.scalar.dma_start(w2f_sb[:], w2f[dtp])
                        w2f_tiles[dtp] = w2f_sb
                    for fl in range(FL):
                        ft = fb * FL + fl
                        for gi, (n0, n1) in enumerate(groups):
                            ps = psA.tile([P, 512], f32, tag="psA")
                            for k in range(KD):
                                nc.tensor.matmul(
                                    ps[:, : n1 - n0],
                                    lhsT=w1_sb[:, k, fl * P : (fl + 1) * P],
                                    rhs=xg[gi][:, k, :],
                                    start=(k == 0),
                                    stop=(k == KD - 1),
                                )
                            nc.scalar.activation(
                                hT[:, ft, n0:n1],
                                ps[:, : n1 - n0],
                                mybir.ActivationFunctionType.Gelu,
                                bias=b1_sb[:, ft : ft + 1],
                            )
                        # fp8-lane tokens: same bf16 GEMM1, but h lands
                        # as e4m3 in DoubleRow pair layout
                        ps = psA.tile([P, 512], f32, tag="psA")
                        for k in range(KD):
                            nc.tensor.matmul(
                                ps[:, :capF],
                                lhsT=w1_sb[:, k, fl * P : (fl + 1) * P],
                                rhs=xf_sb[:, k, :],
                                start=(k == 0),
                                stop=(k == KD - 1),
                            )
                        nc.scalar.activation(
                            hf[:, ft // 2, ft % 2, :],
                            ps[:, :capF],
                            mybir.ActivationFunctionType.Gelu,
                            bias=b1_sb[:, ft : ft + 1],
                        )

            # ---- Phase B: yT = wt * (w2.T @ hT), d-tile per PSUM group ----
            with (
                tc.tile_pool(name="ypool", bufs=3) as ypool,
                tc.tile_pool(name="psB", bufs=3, space="PSUM") as psB,
            ):
                for dt in range(DT):
                    if dt in w2_tiles:
                        w2_sb = w2_tiles.pop(dt)
                        w2f_sb = w2f_tiles.pop(dt)
                    else:
                        w2_sb = w2pool.tile([P, FT, P], bf16, tag="w2sb")
                        nc.scalar.dma_start(w2_sb[:], w2[dt])
                        w2f_sb = w2fpool.tile([P, FTH, 2, P], f8e4, tag="w2fsb")
                        nc.scalar.dma_start(w2f_sb[:], w2f[dt])
                    for n0, n1 in groups:
                        ps = psB.tile([P, 512], f32, tag="psB")
                        for k in range(FT):
                            nc.tensor.matmul(
                                ps[:, : n1 - n0],
                                lhsT=w2_sb[:, k, :],
                                rhs=hT[:, k, n0:n1],
                                start=(k == 0),
                                stop=(k == FT - 1),
                            )
                        yt = ypool.tile([P, 512], f32, tag="yt")
                        nc.vector.tensor_mul(
                            yt[:, : n1 - n0], ps[:, : n1 - n0], wt_sb[:, n0:n1]
                        )
                        # sync HWDGE queue is idle during phase B
                        nc.sync.dma_start(
                            y[dt * P : (dt + 1) * P, n0:n1], yt[:, : n1 - n0]
                        )
                    # fp8 DoubleRow lane: contraction pairs over F
                    ps = psB.tile([P, 512], f32, tag="psB")
                    for kt in range(FTH):
                        nc.tensor.matmul(
                            ps[:, :capF],
                            lhsT=w2f_sb[:, kt, :, :],
                            rhs=hf[:, kt, :, :],
                            start=(kt == 0),
                            stop=(kt == FTH - 1),
                            perf_mode=DR,
                        )
                    yt = ypool.tile([P, 512], f32, tag="yt")
                    nc.vector.tensor_mul(
                        yt[:, :capF], ps[:, :capF], wt_sb[:, capB:cap]
                    )
                    nc.sync.dma_start(
                        y[dt * P : (dt + 1) * P, capB:cap], yt[:, :capF]
                    )

    _split_multi_waits(nc)
    return nc


def _split_multi_waits(nc):
    """The walrus build in this container rejects >1 sync-wait command per
    instruction; hoist extras onto single-wait NOPs just before it."""
    import bass_rust
    import concourse.mybir as mybir

    ctr = 0
    for blk in nc.m.functions[0].blocks:
        insts = blk.instructions
        i = 0
        while i < len(insts):
            inst = insts[i]
            si = inst.sync_info
            if si is None:
                i += 1
                continue
            waits = list(si.on_wait)
            if len(waits) <= 1:
                i += 1
                continue
            for w in waits[:-1]:
                ctr += 1
                nop = bass_rust.InstNoOp(name=f"waitsplit_{ctr}")
                nop.engine = inst.engine
                nop.sync_info = mybir.SyncInfo(on_wait=[w], on_update=[])
                insts.insert(i, nop)
                i += 1
            inst.sync_info = mybir.SyncInfo(
                on_wait=[waits[-1]], on_update=list(si.on_update)
            )
            i += 1


_CACHE = {}


def _get_program(capB, capF):
    key = (capB, capF)
    if key not in _CACHE:
        _CACHE[key] = _build_program(capB, capF)
    return _CACHE[key]


_RUNNER_CACHE = {}


def _make_runner(nc, n_cores=N_CORES):
    """Persistent jitted shard_map over the bass NEFF (one jax.jit per
    program, reused across kernel() calls)."""
    import jax
    from jax.sharding import Mesh, PartitionSpec
    from jax.experimental.shard_map import shard_map
    import concourse.mybir as mybir
    from concourse import bass2jax
    from concourse.bass2jax import _bass_exec_p, partition_id_tensor

    bass2jax.install_neuronx_cc_hook()

    partition_name = nc.partition_id_tensor.name if nc.partition_id_tensor else None
    in_names, out_names, out_avals, zero_shapes = [], [], [], []
    for alloc in nc.m.functions[0].allocations:
        if not isinstance(alloc, mybir.MemoryLocationSet):
            continue
        name = alloc.memorylocations[0].name
        if alloc.kind == "ExternalInput":
            if name != partition_name:
                in_names.append(name)
        elif alloc.kind == "ExternalOutput":
            out_names.append(name)
            shape = tuple(alloc.tensor_shape)
            dtype = mybir.dt.np(alloc.dtype)
            out_avals.append(jax.core.ShapedArray(shape, dtype))
            zero_shapes.append((shape, dtype))
    n_params = len(in_names)
    n_outs = len(out_avals)
    in_names.extend(out_names)
    if partition_name is not None:
        in_names.append(partition_name)

    def _body(*args):
        operands = list(args)
        if partition_name is not None:
            operands.append(partition_id_tensor())
        outs = _bass_exec_p.bind(
            *operands,
            out_avals=tuple(out_avals),
            in_names=tuple(in_names),
            out_names=tuple(out_names),
            lowering_input_output_aliases=(),
            sim_require_finite=True,
            sim_require_nnan=True,
            nc=nc,
        )
        return tuple(outs)

    devices = jax.devices()[:n_cores]
    mesh = Mesh(np.asarray(devices), ("core",))
    in_specs = (PartitionSpec("core"),) * (n_params + n_outs)
    out_specs = (PartitionSpec("core"),) * len(out_names)
    donate = tuple(range(n_params, n_params + n_outs))
    sharded = jax.jit(
        shard_map(
            _body, mesh=mesh, in_specs=in_specs, out_specs=out_specs, check_rep=False
        ),
        donate_argnums=donate,
        keep_unused=True,
    )

    def run(in_maps):
        per_core = [
            [np.asarray(m[name]) for name in in_names[:n_params]] for m in in_maps
        ]
        concat_in = [
            np.concatenate([per_core[c][i] for c in range(n_cores)], axis=0)
            for i in range(n_params)
        ]
        concat_zeros = [
            np.zeros((n_cores * s[0], *s[1:]), dt) for s, dt in zero_shapes
        ]
        out_arrs = sharded(*concat_in, *concat_zeros)
        return [
            {
                name: np.asarray(out_arrs[i]).reshape(
                    n_cores, *out_avals[i].shape
                )[c]
                for i, name in enumerate(out_names)
            }
            for c in range(n_cores)
        ]

    return run


def _get_runner(capB, capF):
    key = (capB, capF)
    if key not in _RUNNER_CACHE:
        _RUNNER_CACHE[key] = _make_runner(_get_program(capB, capF))
    return _RUNNER_CACHE[key]


def prepare(x, gate_w, gate_b, W1, b1, W2, b2):
    """Host routing + per-core input packing."""
    x = np.asarray(x, np.float32)
    B, S, Dx = x.shape
    assert (Dx, W1.shape[2], gate_b.shape[0]) == (D, F, E)
    T = B * S
    flat = x.reshape(T, D)

    top_i, scores = _gate_host(flat, np.asarray(gate_w), np.asarray(gate_b))

    r1s = [np.where(top_i[:, 0] == e)[0] for e in range(E)]
    r2s = [np.where(top_i[:, 1] == e)[0] for e in range(E)]
    # capB sized for the heaviest expert after it sheds KF8 tokens to the
    # fp8 lane; every other expert sheds only what it must (fewer fp8
    # tokens -> less error at identical cost).
    capB = max(
        512,
        -(-max(len(r1s[e]) + len(r2s[e]) - min(KF8, len(r2s[e])) for e in range(E)) // 4)
        * 4,
    )
    bsel = []
    fsel = []
    for e in range(E):
        n_e = len(r1s[e]) + len(r2s[e])
        nf = min(len(r2s[e]), max(0, n_e - capB))
        o = np.argsort(scores[r2s[e], e], kind="stable")
        fsel.append(r2s[e][o[:nf]])
        bsel.append(np.concatenate([r1s[e], r2s[e][o[nf:]]]))
    capF = KF8
    assert max(len(s) for s in fsel) <= capF
    cap = capB + capF

    bf = ml_dtypes.bfloat16
    f8 = ml_dtypes.float8_e4m3
    W1 = np.asarray(W1, np.float32)
    W2 = np.asarray(W2, np.float32)
    b1 = np.asarray(b1, np.float32)

    groups = _token_groups(capB)
    in_maps = []
    for e in range(E):
        n_b, n_f = len(bsel[e]), len(fsel[e])
        # xT: [P, KD, capB];  xT[p, k, t] = x[t, k*128+p]
        xTe = np.zeros((P, KD, capB), bf)
        xb = flat[bsel[e]].astype(bf)  # [n_b, D]
        xTe[:, :, :n_b] = xb.T.reshape(KD, P, n_b).transpose(1, 0, 2)
        xFe = np.zeros((P, KD, capF), bf)
        xf = flat[fsel[e]].astype(bf)
        xFe[:, :, :n_f] = xf.T.reshape(KD, P, n_f).transpose(1, 0, 2)
        # w1: [NFB, P, KD, FBW];  w1[fb, p, k, j] = W1[k*128+p, fb*FBW+j]
        w1p = np.ascontiguousarray(
            W1[e].astype(bf).reshape(KD, P, NFB, FBW).transpose(2, 1, 0, 3)
        )
        # w2: [DT, P, FT, P];  w2[dt, p, f, j] = W2[f*128+p, dt*128+j]
        w2p = np.ascontiguousarray(
            W2[e].astype(bf).reshape(FT, P, DT, P).transpose(2, 1, 0, 3)
        )
        # w2f: [DT, P, FTH, 2, P];  w2f[dt, p, kt, s, j] =
        #   (W2*256)[kt*256 + s*128 + p, dt*128+j] in e4m3
        w2fp = np.ascontiguousarray(
            (W2[e] * F8_SCALE)
            .astype(f8)
            .reshape(FTH, 2, P, DT, P)
            .transpose(3, 2, 0, 1, 4)
        )
        # b1: [P, FT];  b1p[p, f] = b1[f*128+p]
        b1p = np.ascontiguousarray(b1[e].reshape(FT, P).T)
        # wt: [P, cap] broadcast along partitions; fp8 columns carry 1/256
        wte = np.zeros((cap,), np.float32)
        wte[:n_b] = scores[bsel[e], e].astype(np.float32)
        wte[capB : capB + n_f] = scores[fsel[e], e].astype(np.float32) / F8_SCALE
        wtp = np.ascontiguousarray(np.broadcast_to(wte, (P, cap)))
        m = {"w1": w1p, "w2": w2p, "w2f": w2fp, "b1": b1p, "wt": wtp, "xf": xFe}
        for gi, (n0, n1) in enumerate(groups):
            m[f"xg{gi}"] = np.ascontiguousarray(xTe[:, :, n0:n1])
        in_maps.append(m)
    return in_maps, (bsel, fsel), (capB, capF), top_i, scores, (B, S, T)


def combine(results, sels, caps, top_i, scores, b2, shape):
    B, S, T = shape
    bsel, fsel = sels
    capB, capF = caps
    b2 = np.asarray(b2, np.float32)
    out = np.zeros((T, D), np.float32)
    for e in range(E):
        y = results[e]["y"]
        out[bsel[e]] += y[:, : len(bsel[e])].T
        out[fsel[e]] += y[:, capB : capB + len(fsel[e])].T
    if np.any(b2):
        w_dense = np.zeros((T, E), np.float32)
        for k in range(TOP_K):
            w_dense[np.arange(T), top_i[:, k]] += scores[
                np.arange(T), top_i[:, k]
            ].astype(np.float32)
        out += w_dense @ b2
    return out.reshape(B, S, D)


def kernel(x, gate_w, gate_b, W1, b1, W2, b2):
    in_maps, sels, caps, top_i, scores, shape = prepare(
        x, gate_w, gate_b, W1, b1, W2, b2
    )
    results = _get_runner(*caps)(in_maps)
    return combine(results, sels, caps, top_i, scores, b2, shape)
